# revision 1
# baseline (speedup 1.0000x reference)
"""MoE decoder kernel for Trainium2 (8 NeuronCores, expert-parallel).

Strategy
--------
Host (numpy): gate (sigmoid + top-8 + weight normalization), token->expert
dispatch, weight repacking to bf16 in PE-friendly layout, final
scatter-add combine + LayerNorm.

Device (Bass/Tile, SPMD over 8 cores): 8 experts per core.  For each
expert the 4-layer MLP runs with *feature-major* activations
(act^T: [feat, tokens]) so that every matmul uses the natural-layout
weight tile [K=128, M=128] as the stationary operand and the activation
tile [K=128, T] as the moving operand -- no transposes anywhere.
Weights are streamed HBM->SBUF exactly once per expert as large
contiguous "mega chunk" DMAs, double-buffered through a ring pool.

Per (token,expert) pair only selected pairs are computed (capacity =
per-slot max over cores, rounded to 16), so compute ~= the sparse
top-8 workload, and HBM traffic ~= one bf16 read of all expert weights
(the roofline for this problem).
"""

import numpy as np
import ml_dtypes

# problem constants (hardcoded; kernel.py must be self-contained)
B, S, D = 2, 512, 1024
H, BN, O = 2048, 256, 768
E, TOPK = 64, 8
N = B * S
NCORES = 8
EPC = E // NCORES  # experts per core

BF16 = ml_dtypes.bfloat16

LAST_EXEC_NS = None  # test harness reads this after a traced run


# ---------------------------------------------------------------------------
# host-side routing
# ---------------------------------------------------------------------------

def _route(x, gate_w, gate_bias):
    """Replicates the reference gate in float64: returns top_idx [N,8],
    combine weights wc [N,8] (float32)."""
    xf = x.reshape(N, D).astype(np.float64)
    logits = xf @ gate_w.astype(np.float64).T
    scores = 1.0 / (1.0 + np.exp(-logits))
    choice = scores + gate_bias.astype(np.float64)[None, :]
    # top-8, descending, stable (matches jax.lax.top_k tie behavior)
    top_idx = np.argsort(-choice, axis=1, kind="stable")[:, :TOPK]
    top_scores = np.take_along_axis(choice, top_idx, axis=1)
    wc = top_scores / (top_scores.sum(-1, keepdims=True) + 1e-6)
    return top_idx.astype(np.int64), wc.astype(np.float32)


def _assign_experts(counts):
    """Greedy balance: experts -> cores (EPC slots each), sorted desc within
    a core.  Returns assign[core][slot] = expert id."""
    order = np.argsort(-counts, kind="stable")
    loads = [0] * NCORES
    nslot = [0] * NCORES
    assign = [[] for _ in range(NCORES)]
    for e in order:
        # least-loaded core with a free slot
        c = min(
            (c for c in range(NCORES) if nslot[c] < EPC),
            key=lambda c: (loads[c], c),
        )
        assign[c].append(int(e))
        loads[c] += int(counts[e])
        nslot[c] += 1
    return assign  # each list already desc by count (greedy order)


# ---------------------------------------------------------------------------
# device program
# ---------------------------------------------------------------------------

def _build_program(caps):
    import concourse.bass as bass
    import concourse.tile as tile
    from concourse import mybir

    DT = mybir.dt.bfloat16
    F32 = mybir.dt.float32
    SC = int(np.sum(caps))
    offs = np.concatenate([[0], np.cumsum(caps)]).astype(int)

    nc = bass.Bass(trn_type="TRN2")
    w1s = nc.dram_tensor("w1s", [EPC, 4, 128, 4096], DT, kind="ExternalInput")
    w2s = nc.dram_tensor("w2s", [EPC, 8, 128, 4096], DT, kind="ExternalInput")
    w3s = nc.dram_tensor("w3s", [EPC, 4, 128, 1024], DT, kind="ExternalInput")
    w4s = nc.dram_tensor("w4s", [EPC, 128, 1536], DT, kind="ExternalInput")
    xt = nc.dram_tensor("xt", [D, SC], DT, kind="ExternalInput")
    bias = nc.dram_tensor("bias", [128, EPC * 40], F32, kind="ExternalInput")
    out = nc.dram_tensor("out", [O, SC], F32, kind="ExternalOutput")

    GELU = mybir.ActivationFunctionType.Gelu

    with tile.TileContext(nc) as tc:
        with (
            tc.tile_pool(name="wt", bufs=16) as wpool,
            tc.tile_pool(name="xtp", bufs=16) as xpool,
            tc.tile_pool(name="h1p", bufs=32) as h1pool,
            tc.tile_pool(name="h2p", bufs=32) as h2pool,
            tc.tile_pool(name="h3p", bufs=4) as h3pool,
            tc.tile_pool(name="outp", bufs=16) as opool,
            tc.tile_pool(name="ps", bufs=8, space="PSUM") as pspool,
            tc.tile_pool(name="one", bufs=1) as single,
        ):
            bias_sb = single.tile([128, EPC * 40], F32)
            nc.sync.dma_start(out=bias_sb, in_=bias[:, :])
            # Observer ops: ACT and DVE each touch the bias tile once so the
            # bias-DMA tick is already observed by those engines -- keeps every
            # later activation/tensor_scalar at <=1 sync wait (the legacy
            # walrus codegen rejects instructions with 2+ waits).
            obs_a = single.tile([128, 1], F32)
            nc.scalar.copy(out=obs_a, in_=bias_sb[:, 0:1])
            obs_v = single.tile([128, 1], F32)
            nc.vector.tensor_copy(out=obs_v, in_=bias_sb[:, 0:1])

            for r in range(EPC):
                C = int(caps[r])
                off = int(offs[r])
                bcol = r * 40

                # gathered tokens, transposed: 8 k-tiles of [128, C]
                xts = []
                for k in range(8):
                    t = xpool.tile([128, C], DT, tag="xt")
                    nc.sync.dma_start(
                        out=t, in_=xt[k * 128:(k + 1) * 128, off:off + C]
                    )
                    xts.append(t)

                # ---- L1: h1^T[H, C] = gelu(W1^T x + b1), K=D (8 tiles) ----
                h1 = []
                for g in range(2):  # m-groups of 8 feature tiles
                    psums = [pspool.tile([128, C], F32, tag="ps", name=f"ps_{r}_{g}_{_}") for _ in range(8)]
                    for mega in range(2):  # 2 megas x 4 k-chunks
                        wt = wpool.tile([128, 4096], DT, tag="wt")
                        nc.sync.dma_start(out=wt, in_=w1s[r, g * 2 + mega])
                        for c in range(4):
                            k = mega * 4 + c
                            for m in range(8):
                                nc.tensor.matmul(
                                    psums[m],
                                    wt[:, c * 1024 + m * 128: c * 1024 + (m + 1) * 128],
                                    xts[k],
                                    start=(k == 0),
                                    stop=(k == 7),
                                )
                    for m in range(8):
                        hh = h1pool.tile([128, C], DT, tag="h1")
                        nc.scalar.activation(
                            out=hh, in_=psums[m], func=GELU,
                            bias=bias_sb[:, bcol + g * 8 + m: bcol + g * 8 + m + 1],
                        )
                        h1.append(hh)

                # ---- L2: h2^T[H, C] = gelu(W2^T h1 + b2), K=H (16 tiles) ----
                h2 = []
                for g in range(2):
                    psums = [pspool.tile([128, C], F32, tag="ps", name=f"ps_{r}_{g}_{_}") for _ in range(8)]
                    for mega in range(4):
                        wt = wpool.tile([128, 4096], DT, tag="wt")
                        nc.sync.dma_start(out=wt, in_=w2s[r, g * 4 + mega])
                        for c in range(4):
                            k = mega * 4 + c
                            for m in range(8):
                                nc.tensor.matmul(
                                    psums[m],
                                    wt[:, c * 1024 + m * 128: c * 1024 + (m + 1) * 128],
                                    h1[k],
                                    start=(k == 0),
                                    stop=(k == 15),
                                )
                    for m in range(8):
                        hh = h2pool.tile([128, C], DT, tag="h2")
                        nc.scalar.activation(
                            out=hh, in_=psums[m], func=GELU,
                            bias=bias_sb[:, bcol + 16 + g * 8 + m: bcol + 16 + g * 8 + m + 1],
                        )
                        h2.append(hh)

                # ---- L3: h3^T[BN, C] = W3^T h2 + b3, K=H (16 tiles) ----
                psums3 = [pspool.tile([128, C], F32, tag="ps", name=f"ps3_{r}_{_}") for _ in range(2)]
                for mega in range(4):
                    wt = wpool.tile([128, 1024], DT, tag="wt")
                    nc.sync.dma_start(out=wt, in_=w3s[r, mega])
                    for c in range(4):
                        k = mega * 4 + c
                        for m in range(2):
                            nc.tensor.matmul(
                                psums3[m],
                                wt[:, c * 256 + m * 128: c * 256 + (m + 1) * 128],
                                h2[k],
                                start=(k == 0),
                                stop=(k == 15),
                            )
                h3 = []
                for m in range(2):
                    hh = h3pool.tile([128, C], DT, tag="h3")
                    nc.vector.memset(hh, 0)
                    nc.vector.tensor_scalar_add(
                        hh, psums3[m], bias_sb[:, bcol + 32 + m: bcol + 32 + m + 1]
                    )
                    h3.append(hh)

                # ---- L4: out^T[O, C] = W4^T h3 + b4, K=BN (2 tiles) ----
                psums4 = [pspool.tile([128, C], F32, tag="ps", name=f"ps4_{r}_{_}") for _ in range(6)]
                wt = wpool.tile([128, 1536], DT, tag="wt")
                nc.sync.dma_start(out=wt, in_=w4s[r])
                for c in range(2):
                    for m in range(6):
                        nc.tensor.matmul(
                            psums4[m],
                            wt[:, c * 768 + m * 128: c * 768 + (m + 1) * 128],
                            h3[c],
                            start=(c == 0),
                            stop=(c == 1),
                        )
                for m in range(6):
                    ot = opool.tile([128, C], F32, tag="out")
                    nc.vector.memset(ot, 0)
                    nc.vector.tensor_scalar_add(
                        ot, psums4[m], bias_sb[:, bcol + 34 + m: bcol + 34 + m + 1]
                    )
                    nc.sync.dma_start(
                        out=out[m * 128:(m + 1) * 128, off:off + C], in_=ot
                    )

    _legalize_waits(nc, mybir)
    return nc


def _legalize_waits(nc, mybir):
    """The legacy walrus codegen (bass2jax path) rejects instructions carrying
    more than one sync wait.  Split every multi-wait instruction: hoist all
    but the last wait onto same-engine InstNoOp carriers inserted just before
    it (engine program order preserves the gating semantics)."""
    n = 0
    for bb in nc.main_func.blocks:
        insts = bb.instructions
        i = 0
        while i < len(insts):
            ins = insts[i]
            si = ins.sync_info
            if si is not None and si.on_wait and len(si.on_wait) > 1:
                extra = list(si.on_wait[:-1])
                keep = [si.on_wait[-1]]
                for w in extra:
                    noop = mybir.InstNoOp(
                        name=f"NOPW-{n}", engine=ins.engine, ins=[], outs=[],
                        sync_info=mybir.SyncInfo(on_wait=[w], on_update=[]),
                    )
                    n += 1
                    insts.insert(i, noop)
                    i += 1
                ins.sync_info = mybir.SyncInfo(
                    on_wait=keep, on_update=list(si.on_update or [])
                )
            i += 1


# ---------------------------------------------------------------------------
# host-side packing
# ---------------------------------------------------------------------------

def _pack_core(w1, b1, w2, b2, w3, b3, w4, b4, experts):
    """Pack one core's 8 experts into the DRAM layouts the program expects."""
    idx = np.asarray(experts)
    # W1 [e,1024,2048] -> [e,4,128,4096]: chunks (g,k) of [128,1024], 4/mega
    a = w1[idx].reshape(EPC, 8, 128, 2, 1024)
    a = a.transpose(0, 3, 1, 2, 4).reshape(EPC, 16, 128, 1024)
    w1p = np.ascontiguousarray(
        a.reshape(EPC, 4, 4, 128, 1024).transpose(0, 1, 3, 2, 4)
    ).reshape(EPC, 4, 128, 4096).astype(BF16)

    a = w2[idx].reshape(EPC, 16, 128, 2, 1024)
    a = a.transpose(0, 3, 1, 2, 4).reshape(EPC, 32, 128, 1024)
    w2p = np.ascontiguousarray(
        a.reshape(EPC, 8, 4, 128, 1024).transpose(0, 1, 3, 2, 4)
    ).reshape(EPC, 8, 128, 4096).astype(BF16)

    a = w3[idx].reshape(EPC, 16, 128, 256)  # k-chunks of [128,256]
    w3p = np.ascontiguousarray(
        a.reshape(EPC, 4, 4, 128, 256).transpose(0, 1, 3, 2, 4)
    ).reshape(EPC, 4, 128, 1024).astype(BF16)

    a = w4[idx].reshape(EPC, 2, 128, 768)
    w4p = np.ascontiguousarray(a.transpose(0, 2, 1, 3)).reshape(
        EPC, 128, 1536
    ).astype(BF16)

    # biases: per expert 40 cols of [128]: L1 m0-15 | L2 m0-15 | L3 m0-1 | L4 m0-5
    bb = np.concatenate(
        [
            b1[idx].reshape(EPC, 16, 128),
            b2[idx].reshape(EPC, 16, 128),
            b3[idx].reshape(EPC, 2, 128),
            b4[idx].reshape(EPC, 6, 128),
        ],
        axis=1,
    )  # [EPC, 40, 128]
    biasp = np.ascontiguousarray(
        bb.reshape(EPC * 40, 128).T
    ).astype(np.float32)  # [128, EPC*40]
    return w1p, w2p, w3p, w4p, biasp


def kernel(x, gate_w, gate_bias, w1, b1, w2, b2, w3, b3, w4, b4, ln_w, ln_b):
    global LAST_EXEC_NS
    x = np.asarray(x, np.float32)
    xf = x.reshape(N, D)

    top_idx, wc = _route(x, np.asarray(gate_w, np.float32),
                         np.asarray(gate_bias, np.float32))

    # token lists per expert
    counts = np.bincount(top_idx.ravel(), minlength=E)
    tok_of = [[] for _ in range(E)]
    w_of = [[] for _ in range(E)]
    flat_tok = np.repeat(np.arange(N), TOPK)
    flat_exp = top_idx.ravel()
    flat_w = wc.ravel()
    order = np.argsort(flat_exp, kind="stable")
    for t, e, w in zip(flat_tok[order], flat_exp[order], flat_w[order]):
        tok_of[e].append(int(t))
        w_of[e].append(float(w))

    assign = _assign_experts(counts)

    # per-slot capacities (shared across cores; slots sorted desc by count)
    caps = np.zeros(EPC, int)
    for c in range(NCORES):
        for r, e in enumerate(assign[c]):
            caps[r] = max(caps[r], counts[e])
    caps = ((caps + 15) // 16) * 16
    SC = int(caps.sum())
    offs = np.concatenate([[0], np.cumsum(caps)]).astype(int)

    nc = _build_program(caps)

    w1a = np.asarray(w1, np.float32); b1a = np.asarray(b1, np.float32)
    w2a = np.asarray(w2, np.float32); b2a = np.asarray(b2, np.float32)
    w3a = np.asarray(w3, np.float32); b3a = np.asarray(b3, np.float32)
    w4a = np.asarray(w4, np.float32); b4a = np.asarray(b4, np.float32)

    xt_bf = xf.T.astype(BF16)  # [D, N]
    in_maps = []
    for c in range(NCORES):
        w1p, w2p, w3p, w4p, biasp = _pack_core(
            w1a, b1a, w2a, b2a, w3a, b3a, w4a, b4a, assign[c]
        )
        xtc = np.zeros((D, SC), BF16)
        for r, e in enumerate(assign[c]):
            ids = tok_of[e]
            xtc[:, offs[r]:offs[r] + len(ids)] = xt_bf[:, ids]
        in_maps.append(
            {"w1s": w1p, "w2s": w2p, "w3s": w3p, "w4s": w4p,
             "xt": xtc, "bias": biasp}
        )

    from concourse.bass_utils import run_bass_kernel_spmd

    res = run_bass_kernel_spmd(nc, in_maps, core_ids=list(range(NCORES)))
    LAST_EXEC_NS = res.exec_time_ns

    # combine: scatter-add weighted expert outputs (float64 accum)
    combined = np.zeros((N, O), np.float64)
    for c in range(NCORES):
        yc = np.asarray(res.results[c]["out"], np.float32)  # [O, SC]
        for r, e in enumerate(assign[c]):
            ids = tok_of[e]
            if not ids:
                continue
            wv = np.asarray(w_of[e], np.float64)
            y = yc[:, offs[r]:offs[r] + len(ids)].astype(np.float64)
            np.add.at(combined, ids, (y * wv[None, :]).T)

    combined = combined.astype(np.float32)
    mu = combined.mean(-1, keepdims=True)
    var = combined.var(-1, keepdims=True)
    outn = (combined - mu) / np.sqrt(var + 1e-5)
    outn = outn * np.asarray(ln_w, np.float32) + np.asarray(ln_b, np.float32)
    return outn.reshape(B, S, O).astype(np.float32)



# revision 3
# speedup vs baseline: 1.2461x; 1.2461x over previous
"""MoE decoder kernel for Trainium2 (8 NeuronCores, expert-parallel).

Strategy
--------
Host (numpy): gate (sigmoid + top-8 + weight normalization), token->expert
dispatch, weight repacking to bf16 in PE-friendly layout, final
scatter-add combine + LayerNorm.

Device (Bass/Tile, SPMD over 8 cores): 8 experts per core.  For each
expert the 4-layer MLP runs with *feature-major* activations
(act^T: [feat, tokens]) so that every matmul uses the natural-layout
weight tile [K=128, M=128] as the stationary operand and the activation
tile [K=128, T] as the moving operand -- no transposes anywhere.

DMA queue discipline (the v2 speedup): the SP-engine HWDGE queue
(qSyncDynamicHW) carries ONLY the large weight streams (8 KiB+ lines,
~420 GB/s sustained).  Token gathers, bias, and output stores ride the
Activation-engine HWDGE queue so a store that waits on compute can
never stall weight prefetch in the FIFO.  W3+W4 are merged into one
[128, 5632] transfer; token/output tiles are packed host-side so each
expert needs exactly one gather DMA and one store DMA.
"""

import numpy as np
import ml_dtypes

# problem constants (hardcoded; kernel.py must be self-contained)
B, S, D = 2, 512, 1024
H, BN, O = 2048, 256, 768
E, TOPK = 64, 8
N = B * S
NCORES = 8
EPC = E // NCORES  # experts per core

BF16 = ml_dtypes.bfloat16

LAST_EXEC_NS = None  # test harness reads this after a traced run


# ---------------------------------------------------------------------------
# host-side routing
# ---------------------------------------------------------------------------

def _route(x, gate_w, gate_bias):
    """Replicates the reference gate in float64: returns top_idx [N,8],
    combine weights wc [N,8] (float32)."""
    xf = x.reshape(N, D).astype(np.float64)
    logits = xf @ gate_w.astype(np.float64).T
    scores = 1.0 / (1.0 + np.exp(-logits))
    choice = scores + gate_bias.astype(np.float64)[None, :]
    # top-8, descending, stable (matches jax.lax.top_k tie behavior)
    top_idx = np.argsort(-choice, axis=1, kind="stable")[:, :TOPK]
    top_scores = np.take_along_axis(choice, top_idx, axis=1)
    wc = top_scores / (top_scores.sum(-1, keepdims=True) + 1e-6)
    return top_idx.astype(np.int64), wc.astype(np.float32)


def _assign_experts(counts):
    """Greedy balance: experts -> cores (EPC slots each), sorted desc within
    a core.  Returns assign[core][slot] = expert id."""
    order = np.argsort(-counts, kind="stable")
    loads = [0] * NCORES
    nslot = [0] * NCORES
    assign = [[] for _ in range(NCORES)]
    for e in order:
        # least-loaded core with a free slot
        c = min(
            (c for c in range(NCORES) if nslot[c] < EPC),
            key=lambda c: (loads[c], c),
        )
        assign[c].append(int(e))
        loads[c] += int(counts[e])
        nslot[c] += 1
    return assign  # each list already desc by count (greedy order)


# ---------------------------------------------------------------------------
# device program
# ---------------------------------------------------------------------------

def _build_program(caps):
    import concourse.bass as bass
    import concourse.tile as tile
    from concourse import mybir

    DT = mybir.dt.bfloat16
    F32 = mybir.dt.float32
    SC = int(np.sum(caps))
    offs = np.concatenate([[0], np.cumsum(caps)]).astype(int)

    nc = bass.Bass(trn_type="TRN2")
    w1s = nc.dram_tensor("w1s", [EPC, 4, 128, 4096], DT, kind="ExternalInput")
    w2s = nc.dram_tensor("w2s", [EPC, 8, 128, 4096], DT, kind="ExternalInput")
    w34s = nc.dram_tensor("w34s", [EPC, 128, 5632], DT, kind="ExternalInput")
    xt = nc.dram_tensor("xt", [128, 8 * SC], DT, kind="ExternalInput")
    bias = nc.dram_tensor("bias", [128, EPC * 40], F32, kind="ExternalInput")
    out = nc.dram_tensor("out", [128, 6 * SC], F32, kind="ExternalOutput")

    GELU = mybir.ActivationFunctionType.Gelu

    with tile.TileContext(nc) as tc:
        with (
            tc.tile_pool(name="wt", bufs=14) as wpool,
            tc.tile_pool(name="w34", bufs=2) as w34pool,
            tc.tile_pool(name="xtp", bufs=3) as xpool,
            tc.tile_pool(name="h1p", bufs=32) as h1pool,
            tc.tile_pool(name="h2p", bufs=32) as h2pool,
            tc.tile_pool(name="h3p", bufs=4) as h3pool,
            tc.tile_pool(name="outp", bufs=3) as opool,
            tc.tile_pool(name="ps", bufs=8, space="PSUM") as pspool,
            tc.tile_pool(name="one", bufs=1) as single,
        ):
            bias_sb = single.tile([128, EPC * 40], F32)
            nc.scalar.dma_start(out=bias_sb, in_=bias[:, :])
            # Observer ops: ACT and DVE each touch the bias tile once so the
            # bias-DMA tick is already observed by those engines -- keeps every
            # later activation/tensor_scalar at <=1 sync wait (the legacy
            # walrus codegen rejects instructions with 2+ waits).
            obs_a = single.tile([128, 1], F32)
            nc.scalar.copy(out=obs_a, in_=bias_sb[:, 0:1])
            obs_v = single.tile([128, 1], F32)
            nc.vector.tensor_copy(out=obs_v, in_=bias_sb[:, 0:1])

            for r in range(EPC):
                C = int(caps[r])
                off = int(offs[r])
                bcol = r * 40

                # gathered tokens: one [128, 8*C] tile; k-tile k at cols k*C
                xtile = xpool.tile([128, 8 * C], DT, tag="xt")
                nc.scalar.dma_start(
                    out=xtile, in_=xt[:, 8 * off:8 * off + 8 * C]
                )

                # ---- L1: h1^T[H, C] = gelu(W1^T x + b1), K=D (8 tiles) ----
                h1 = []
                for g in range(2):  # m-groups of 8 feature tiles
                    psums = [pspool.tile([128, C], F32, tag="ps", name=f"ps_{r}_{g}_{_}") for _ in range(8)]
                    for mega in range(2):  # 2 megas x 4 k-chunks
                        wt = wpool.tile([128, 4096], DT, tag="wt")
                        nc.sync.dma_start(out=wt, in_=w1s[r, g * 2 + mega])
                        for c in range(4):
                            k = mega * 4 + c
                            for m in range(8):
                                nc.tensor.matmul(
                                    psums[m],
                                    wt[:, c * 1024 + m * 128: c * 1024 + (m + 1) * 128],
                                    xtile[:, k * C:(k + 1) * C],
                                    start=(k == 0),
                                    stop=(k == 7),
                                )
                    for m in range(8):
                        hh = h1pool.tile([128, C], DT, tag="h1")
                        nc.scalar.activation(
                            out=hh, in_=psums[m], func=GELU,
                            bias=bias_sb[:, bcol + g * 8 + m: bcol + g * 8 + m + 1],
                        )
                        h1.append(hh)

                # ---- L2: h2^T[H, C] = gelu(W2^T h1 + b2), K=H (16 tiles) ----
                h2 = []
                for g in range(2):
                    psums = [pspool.tile([128, C], F32, tag="ps", name=f"ps_{r}_{g}_{_}") for _ in range(8)]
                    for mega in range(4):
                        wt = wpool.tile([128, 4096], DT, tag="wt")
                        nc.sync.dma_start(out=wt, in_=w2s[r, g * 4 + mega])
                        for c in range(4):
                            k = mega * 4 + c
                            for m in range(8):
                                nc.tensor.matmul(
                                    psums[m],
                                    wt[:, c * 1024 + m * 128: c * 1024 + (m + 1) * 128],
                                    h1[k],
                                    start=(k == 0),
                                    stop=(k == 15),
                                )
                    for m in range(8):
                        hh = h2pool.tile([128, C], DT, tag="h2")
                        nc.scalar.activation(
                            out=hh, in_=psums[m], func=GELU,
                            bias=bias_sb[:, bcol + 16 + g * 8 + m: bcol + 16 + g * 8 + m + 1],
                        )
                        h2.append(hh)

                # ---- L3+L4 weights: one [128, 5632] transfer ----
                wt34 = w34pool.tile([128, 5632], DT, tag="w34")
                nc.sync.dma_start(out=wt34, in_=w34s[r])

                # ---- L3: h3^T[BN, C] = W3^T h2 + b3, K=H (16 k-chunks) ----
                psums3 = [pspool.tile([128, C], F32, tag="ps", name=f"ps3_{r}_{_}") for _ in range(2)]
                for k in range(16):
                    for m in range(2):
                        nc.tensor.matmul(
                            psums3[m],
                            wt34[:, k * 256 + m * 128: k * 256 + (m + 1) * 128],
                            h2[k],
                            start=(k == 0),
                            stop=(k == 15),
                        )
                h3 = []
                for m in range(2):
                    hh = h3pool.tile([128, C], DT, tag="h3")
                    nc.vector.tensor_scalar_add(
                        hh, psums3[m], bias_sb[:, bcol + 32 + m: bcol + 32 + m + 1]
                    )
                    h3.append(hh)

                # ---- L4: out^T[O, C] = W4^T h3 + b4, K=BN (2 tiles) ----
                psums4 = [pspool.tile([128, C], F32, tag="ps", name=f"ps4_{r}_{_}") for _ in range(6)]
                for c in range(2):
                    for m in range(6):
                        nc.tensor.matmul(
                            psums4[m],
                            wt34[:, 4096 + c * 768 + m * 128: 4096 + c * 768 + (m + 1) * 128],
                            h3[c],
                            start=(c == 0),
                            stop=(c == 1),
                        )
                ot = opool.tile([128, 6 * C], F32, tag="out")
                for m in range(6):
                    nc.vector.tensor_scalar_add(
                        ot[:, m * C:(m + 1) * C], psums4[m],
                        bias_sb[:, bcol + 34 + m: bcol + 34 + m + 1],
                    )
                nc.scalar.dma_start(
                    out=out[:, 6 * off:6 * off + 6 * C], in_=ot
                )

    _legalize_waits(nc, mybir)
    return nc


def _legalize_waits(nc, mybir):
    """The legacy walrus codegen (bass2jax path) rejects instructions carrying
    more than one sync wait.  Split every multi-wait instruction: hoist all
    but the last wait onto same-engine InstNoOp carriers inserted just before
    it (engine program order preserves the gating semantics)."""
    n = 0
    for bb in nc.main_func.blocks:
        insts = bb.instructions
        i = 0
        while i < len(insts):
            ins = insts[i]
            si = ins.sync_info
            if si is not None and si.on_wait and len(si.on_wait) > 1:
                extra = list(si.on_wait[:-1])
                keep = [si.on_wait[-1]]
                for w in extra:
                    noop = mybir.InstNoOp(
                        name=f"NOPW-{n}", engine=ins.engine, ins=[], outs=[],
                        sync_info=mybir.SyncInfo(on_wait=[w], on_update=[]),
                    )
                    n += 1
                    insts.insert(i, noop)
                    i += 1
                ins.sync_info = mybir.SyncInfo(
                    on_wait=keep, on_update=list(si.on_update or [])
                )
            i += 1


# ---------------------------------------------------------------------------
# host-side packing
# ---------------------------------------------------------------------------

def _pack_core(w1, b1, w2, b2, w3, b3, w4, b4, experts):
    """Pack one core's 8 experts into the DRAM layouts the program expects."""
    idx = np.asarray(experts)
    # W1 [e,1024,2048] -> [e,4,128,4096]: chunks (g,k) of [128,1024], 4/mega
    a = w1[idx].reshape(EPC, 8, 128, 2, 1024)
    a = a.transpose(0, 3, 1, 2, 4).reshape(EPC, 16, 128, 1024)
    w1p = np.ascontiguousarray(
        a.reshape(EPC, 4, 4, 128, 1024).transpose(0, 1, 3, 2, 4)
    ).reshape(EPC, 4, 128, 4096).astype(BF16)

    a = w2[idx].reshape(EPC, 16, 128, 2, 1024)
    a = a.transpose(0, 3, 1, 2, 4).reshape(EPC, 32, 128, 1024)
    w2p = np.ascontiguousarray(
        a.reshape(EPC, 8, 4, 128, 1024).transpose(0, 1, 3, 2, 4)
    ).reshape(EPC, 8, 128, 4096).astype(BF16)

    # W3 [e,2048,256] -> [128, 16*256]: k-chunk k at cols k*256
    a = w3[idx].reshape(EPC, 16, 128, 256)
    w3p = a.transpose(0, 2, 1, 3).reshape(EPC, 128, 4096)
    # W4 [e,256,768] -> [128, 2*768]: k-chunk c at cols c*768
    a = w4[idx].reshape(EPC, 2, 128, 768)
    w4p = a.transpose(0, 2, 1, 3).reshape(EPC, 128, 1536)
    w34p = np.ascontiguousarray(
        np.concatenate([w3p, w4p], axis=2)
    ).astype(BF16)  # [EPC, 128, 5632]

    # biases: per expert 40 cols of [128]: L1 m0-15 | L2 m0-15 | L3 m0-1 | L4 m0-5
    bb = np.concatenate(
        [
            b1[idx].reshape(EPC, 16, 128),
            b2[idx].reshape(EPC, 16, 128),
            b3[idx].reshape(EPC, 2, 128),
            b4[idx].reshape(EPC, 6, 128),
        ],
        axis=1,
    )  # [EPC, 40, 128]
    biasp = np.ascontiguousarray(
        bb.reshape(EPC * 40, 128).T
    ).astype(np.float32)  # [128, EPC*40]
    return w1p, w2p, w34p, biasp


def kernel(x, gate_w, gate_bias, w1, b1, w2, b2, w3, b3, w4, b4, ln_w, ln_b):
    global LAST_EXEC_NS
    x = np.asarray(x, np.float32)
    xf = x.reshape(N, D)

    top_idx, wc = _route(x, np.asarray(gate_w, np.float32),
                         np.asarray(gate_bias, np.float32))

    # token lists per expert
    counts = np.bincount(top_idx.ravel(), minlength=E)
    tok_of = [[] for _ in range(E)]
    w_of = [[] for _ in range(E)]
    flat_tok = np.repeat(np.arange(N), TOPK)
    flat_exp = top_idx.ravel()
    flat_w = wc.ravel()
    order = np.argsort(flat_exp, kind="stable")
    for t, e, w in zip(flat_tok[order], flat_exp[order], flat_w[order]):
        tok_of[e].append(int(t))
        w_of[e].append(float(w))

    assign = _assign_experts(counts)

    # per-slot capacities (shared across cores; slots sorted desc by count)
    caps = np.zeros(EPC, int)
    for c in range(NCORES):
        for r, e in enumerate(assign[c]):
            caps[r] = max(caps[r], counts[e])
    caps = ((caps + 15) // 16) * 16
    SC = int(caps.sum())
    offs = np.concatenate([[0], np.cumsum(caps)]).astype(int)

    nc = _build_program(caps)

    w1a = np.asarray(w1, np.float32); b1a = np.asarray(b1, np.float32)
    w2a = np.asarray(w2, np.float32); b2a = np.asarray(b2, np.float32)
    w3a = np.asarray(w3, np.float32); b3a = np.asarray(b3, np.float32)
    w4a = np.asarray(w4, np.float32); b4a = np.asarray(b4, np.float32)

    xt_bf = xf.T.astype(BF16)  # [D, N]
    in_maps = []
    for c in range(NCORES):
        w1p, w2p, w34p, biasp = _pack_core(
            w1a, b1a, w2a, b2a, w3a, b3a, w4a, b4a, assign[c]
        )
        # token gather, packed: expert slot r at cols 8*off, k-tile k at +k*C
        xtc = np.zeros((128, 8 * SC), BF16)
        for r, e in enumerate(assign[c]):
            ids = tok_of[e]
            Cr = int(caps[r])
            o8 = 8 * offs[r]
            for k in range(8):
                xtc[:, o8 + k * Cr: o8 + k * Cr + len(ids)] = (
                    xt_bf[k * 128:(k + 1) * 128, ids]
                )
        in_maps.append(
            {"w1s": w1p, "w2s": w2p, "w34s": w34p,
             "xt": xtc, "bias": biasp}
        )

    from concourse.bass_utils import run_bass_kernel_spmd

    res = run_bass_kernel_spmd(nc, in_maps, core_ids=list(range(NCORES)))
    LAST_EXEC_NS = res.exec_time_ns

    # combine: scatter-add weighted expert outputs (float64 accum)
    combined = np.zeros((N, O), np.float64)
    for c in range(NCORES):
        yc = np.asarray(res.results[c]["out"], np.float32)  # [128, 6*SC]
        for r, e in enumerate(assign[c]):
            ids = tok_of[e]
            if not ids:
                continue
            Cr = int(caps[r])
            o6 = 6 * offs[r]
            wv = np.asarray(w_of[e], np.float64)
            # y[m*128+p, j] = yc[p, o6 + m*Cr + j]
            y = yc[:, o6:o6 + 6 * Cr].reshape(128, 6, Cr)[:, :, :len(ids)]
            y = y.transpose(1, 0, 2).reshape(O, len(ids)).astype(np.float64)
            np.add.at(combined, ids, (y * wv[None, :]).T)

    combined = combined.astype(np.float32)
    mu = combined.mean(-1, keepdims=True)
    var = combined.var(-1, keepdims=True)
    outn = (combined - mu) / np.sqrt(var + 1e-5)
    outn = outn * np.asarray(ln_w, np.float32) + np.asarray(ln_b, np.float32)
    return outn.reshape(B, S, O).astype(np.float32)
